# revision 50
# baseline (speedup 1.0000x reference)
"""Causal attention (B=4, S=2048, D=1024, single head) on 8 TRN2 NeuronCores.

Sharding: data-parallel over batch (4 pairs of cores); within each pair
the K/V context is split by interleaved 128-row chunks (core parity p
owns global k-chunks {2j+p}).

Algebraic folding: scores = (x Wq^T)(x Wk^T)^T = x (Wq^T Wk) x^T, so the
host folds M = Wq^T Wk once (weight-only precompute) and the device
projects qm = x M for its own 1024 rows; the raw x^T it already holds
serves directly as the score lhsT (the K projection disappears).

Gathered q order is PER-CORE [own 8 blocks | partner 8 blocks], so score
tiles 0/1 (own q rows) depend only on locally projected qm and provide
~46us of PE cover in front of the pairwise AllGather that delivers the
partner's qm.  AllGather concatenates by rank (not by "peer"), which is
rank-asymmetric; each core recovers the partner's half SPMD-uniformly as
(rank0_half + rank1_half) - own_half on GpSimd.  Causal block structure
(njs, masked tiles) is identical in this order for both parities; all
parity-dependent structure lives in input data (per-core column-permuted
x, per-core mask tiles) and in the host-side unscramble.

Each core computes its causal score blocks against its own context and
produces *unnormalized* partial attention output plus the per-row
partial softmax denominator.  The host maps both cores' partials back to
global row order, adds them, and normalizes.

All matmuls run in bf16 (fp32 PSUM accumulation); inputs are pre-cast on
the host.
"""

import sys

if "/opt/trn_rl_repo" not in sys.path:
    sys.path.insert(0, "/opt/trn_rl_repo")

import ml_dtypes
import numpy as np

import concourse.bacc as bacc
import concourse.tile as tile
from concourse import mybir
from concourse.bass_utils import run_bass_kernel_spmd

# bass_utils imports antenv.axon_hooks when tracing is requested (e.g. via a
# BASS_TRACE env var); the image's antenv lacks that module, so provide a
# no-op fallback rather than crashing.
try:
    import antenv.axon_hooks  # noqa: F401
except ImportError:
    import types as _types

    _ah = _types.ModuleType("antenv.axon_hooks")
    _ah._hook = None
    _ah.set_axon_ntff_profile_hook = lambda h: setattr(_ah, "_hook", h)
    _ah.get_axon_ntff_profile_hook = lambda: _ah._hook
    sys.modules["antenv.axon_hooks"] = _ah

B, S, D = 4, 2048, 1024
NB = S // 128          # 16 q-blocks of 128 per batch
NT = S // 512          # 4 q-tiles of 512
IC = D // 128          # 8 contraction chunks
OC = D // 128          # 8 output-dim chunks
LC = 8                 # local k-chunks per core (S/2/128)
NMSK = 16              # mask tiles: 4 per q-tile
SCALE = 1.0 / np.sqrt(D)  # 0.03125
NJ_TILE = [4, 8, 4, 8]  # local k-chunks needed per gathered q-tile

BF16 = mybir.dt.bfloat16
F32 = mybir.dt.float32

_module_cache = None
last_results = None  # BassKernelResults of the most recent run (for test harness)


def _masked_js(tt):
    """Local chunk indices whose score blocks need a mask for q-tile tt."""
    return range(4) if tt in (0, 2) else range(4, 8)


def _build_module():
    nc = bacc.Bacc("TRN2", target_bir_lowering=False, debug=False, num_devices=8)
    # x arrives as contiguous (i, half) blocks of [128, 512]
    xT = nc.dram_tensor("xT", [IC * 2 * 128, 512], BF16, kind="ExternalInput").ap()
    wm = nc.dram_tensor("wm", [IC * 128, D], BF16, kind="ExternalInput").ap()
    wvT = nc.dram_tensor("wvT", [D, D], BF16, kind="ExternalInput").ap()
    msk = nc.dram_tensor("msk", [NMSK * 128, 512], BF16, kind="ExternalInput").ap()
    out_p = nc.dram_tensor("out_p", [S, D], F32, kind="ExternalOutput").ap()
    rs_out = nc.dram_tensor("rs_out", [1, S], F32, kind="ExternalOutput").ap()

    with tile.TileContext(nc) as tc:
        with (
            tc.tile_pool(name="wp", bufs=1) as wp,
            tc.tile_pool(name="xp", bufs=1) as xp,
            tc.tile_pool(name="kqv", bufs=1) as kqv,
            tc.tile_pool(name="mp", bufs=1) as mp,
            tc.tile_pool(name="ptp", bufs=2) as ptp,
            tc.tile_pool(name="stg", bufs=4) as stg,
        ):
            xt_sb = [
                xp.tile([128, S // 2], BF16, tag=f"x{i}", name=f"x{i}")
                for i in range(IC)
            ]
            wm_sb = [
                wp.tile([128, D], BF16, tag=f"wm{i}", name=f"wm{i}") for i in range(IC)
            ]
            wv_sb = [
                wp.tile([128, D], BF16, tag=f"wv{i}", name=f"wv{i}") for i in range(IC)
            ]
            # wave 1: x first half + M (one whole-tile DMA per chunk so the
            # i=0 projection chain unblocks after a single transfer)
            for i in range(IC):
                nc.sync.dma_start(
                    xt_sb[i][:, 0:512], xT[128 * 2 * i : 128 * (2 * i + 1), :]
                )
            for i in range(IC):
                nc.scalar.dma_start(wm_sb[i], wm[128 * i : 128 * (i + 1), :])
            # wave 2: x second half + Wv
            for i in range(IC):
                nc.sync.dma_start(
                    xt_sb[i][:, 512:1024], xT[128 * (2 * i + 1) : 128 * (2 * i + 2), :]
                )
            for i in range(IC):
                nc.scalar.dma_start(wv_sb[i], wvT[128 * i : 128 * (i + 1), :])
            # wave 3: masks
            mask_all = mp.tile([128, NMSK, 512], BF16, tag="masks", name="masks")
            nc.scalar.dma_start(mask_all, msk.rearrange("(m p) c -> p m c", p=128))
            ones_sb = mp.tile([128, 1], BF16, tag="ones", name="ones")
            nc.any.memset(ones_sb, 1.0)

            # gathered qm piece tiles, one per 512-col q-tile, indexed
            # [o%2 (even/odd), o//2, col].  qp[0]/qp[1]: own halves;
            # qp[2]/qp[3]: partner pieces, recovered from the AllGathers as
            # (rank0_half + rank1_half) - own_half
            qp = [
                kqv.tile([128, 2, OC // 2, 512], BF16, tag=f"qp{t}", name=f"qp{t}")
                for t in range(NT)
            ]
            qb = [
                [
                    kqv.tile(
                        [128, 2, OC // 2, 512], BF16,
                        tag=f"qb{st}{r}", name=f"qb{st}{r}",
                    )
                    for r in range(2)
                ]
                for st in range(2)
            ]
            vn_sb = [kqv.tile([128, D], BF16, tag=f"vn{j}", name=f"vn{j}") for j in range(LC)]

            # DRAM bounce buffers, one pair per exchanged piece.
            # Row layout: [4 even-o blocks | 4 odd-o blocks] x 128.
            qhalf = [
                nc.dram_tensor(f"qhalf{st}", [D, 512], BF16, kind="Internal").ap()
                for st in range(2)
            ]
            qfull = [
                nc.dram_tensor(f"qfull{st}", [2 * D, 512], BF16, kind="Internal").ap()
                for st in range(2)
            ]

            # ---- phase 1 projections: i-outer chains across 8 PSUM banks so
            #      the PE starts as soon as the first (w, x) chunks land ----
            def proj_iouter(ps1, lhs_slices, rhs_slices, dsts, pname, order="i"):
                """order='i': i-outer (all 8 chains advance together; best for
                the first wave, whose inputs stream in chunk by chunk).
                order='o': o-outer (chains complete one at a time, so each
                PSUM bank's drain-cast overlaps the next chain instead of all
                casts bunching after the final i-wave)."""
                pps = [
                    ps1.tile([128, 512], F32, tag=f"proj8_{o}", bufs=1, name=f"{pname}{o}")
                    for o in range(len(dsts))
                ]
                if order == "i":
                    for i in range(IC):
                        for o in range(len(dsts)):
                            nc.tensor.matmul(
                                pps[o],
                                lhsT=lhs_slices(i, o),
                                rhs=rhs_slices(i, o),
                                start=(i == 0),
                                stop=(i == IC - 1),
                            )
                    for o, dst in enumerate(dsts):
                        dst(pps[o])
                else:
                    for o in range(len(dsts)):
                        for i in range(IC):
                            nc.tensor.matmul(
                                pps[o],
                                lhsT=lhs_slices(i, o),
                                rhs=rhs_slices(i, o),
                                start=(i == 0),
                                stop=(i == IC - 1),
                            )
                        dsts[o](pps[o])

            def copy_to(dst, o=0):
                # phase-1 copies alternate DVE / Scalar so the 8 PSUM->SBUF
                # casts of a projection wave drain in ~half the serial time
                # (GpSimd cannot read PSUM)
                if o % 2 == 0:
                    return lambda pp: nc.vector.tensor_copy(dst, pp)
                return lambda pp: nc.scalar.copy(dst, pp)

            def qm_proj(ps1, rhs, dst_qp, pname, order="i"):
                proj_iouter(
                    ps1,
                    lambda i, o: wm_sb[i][:, 128 * o : 128 * (o + 1)],
                    lambda i, o: rhs(i),
                    [
                        copy_to(dst_qp[:, o % 2, o // 2, :], o)
                        for o in range(OC)
                    ],
                    pname,
                    order=order,
                )

            def exchange(st):
                """AllGather own piece st across the pair (AllReduce measures
                ~2x slower on the CC engines); the partner's piece is then
                (rank0 + rank1) - own, rank-symmetric."""
                # piece 0's stores on sync, piece 1's on scalar: piece 1's
                # must not queue behind piece 0's AllGather-gated loads
                eng = nc.sync if st == 0 else nc.scalar
                for h in range(2):
                    eng.dma_start(
                        qhalf[st][512 * h : 512 * (h + 1), :].rearrange(
                            "(o p) c -> p o c", p=128
                        ),
                        qp[st][:, h, :, :],
                    )
                nc.gpsimd.collective_compute(
                    kind="AllGather",
                    op=mybir.AluOpType.bypass,
                    replica_groups=[[0, 1], [2, 3], [4, 5], [6, 7]],
                    ins=[qhalf[st]],
                    outs=[qfull[st]],
                )
                for r in range(2):
                    nc.sync.dma_start(
                        qb[st][r],
                        qfull[st][1024 * r : 1024 * (r + 1), :].rearrange(
                            "(h o p) c -> p h o c", p=128, h=2
                        ),
                    )

            def recover(st):
                """partner piece st = (rank0 + rank1) - own, on DVE.  Emitted
                mid-phase-2, with a wait-until hint so the static DVE order
                places these after the preceding tile's copies -- the sim
                models the post-collective loads optimistically and would
                otherwise hoist these gated ops ahead of ready work (GpSimd
                tensor ops are ~9.5us each -- far too slow)."""
                with tc.tile_wait_until(0.085 + 0.008 * st):
                    nc.vector.tensor_add(qp[2 + st], qb[st][0], qb[st][1])
                with tc.tile_wait_until(0.087 + 0.008 * st):
                    nc.vector.tensor_sub(qp[2 + st], qp[2 + st], qp[st])

            with tc.tile_pool(name="ps1", bufs=1, space="PSUM") as ps1:
                # own qm pieces 0 and 1; each is exchanged as soon as its
                # casts land, and the partner pieces are only needed by
                # attention tiles 2/3 (~50us of PE cover)
                qm_proj(ps1, lambda i: xt_sb[i][:, 0:512], qp[0], "pq0")
                exchange(0)
                qm_proj(ps1, lambda i: xt_sb[i][:, 512:1024], qp[1], "pq1", order="o")
                exchange(1)
                # V projection for chunks j<4 (all that attention tiles 0 and
                # 2 need); chunks j>=4 are projected in vn_late before tile 1
                proj_iouter(
                    ps1,
                    lambda i, c: xt_sb[i][:, 128 * (c // 2) : 128 * (c // 2 + 1)],
                    lambda i, c: wv_sb[i][:, 512 * (c % 2) : 512 * (c % 2 + 1)],
                    [
                        (lambda dst: lambda pp: nc.vector.tensor_copy(dst, pp))(
                            vn_sb[c // 2][:, 512 * (c % 2) : 512 * (c % 2 + 1)]
                        )
                        for c in range(8)
                    ],
                    "pva",
                    order="o",
                )

            # ---- phase 2: attention over gathered q-tiles.  The raw x^T in
            #      SBUF is the score lhsT (k == x after the M folding). ----
            rs_sb = mp.tile([1, S], F32, tag="rs", name="rs")
            with tc.tile_pool(name="ps2", bufs=2, space="PSUM") as ps:

                def vn_late():
                    for c in range(8):
                        j, ot = 4 + c // 2, c % 2
                        pp = ps.tile([128, 512], F32, tag="score", bufs=4, name="pvb")
                        for i in range(IC):
                            nc.tensor.matmul(
                                pp,
                                lhsT=xt_sb[i][:, 128 * j : 128 * (j + 1)],
                                rhs=wv_sb[i][:, 512 * ot : 512 * (ot + 1)],
                                start=(i == 0),
                                stop=(i == IC - 1),
                            )
                        nc.vector.tensor_copy(
                            vn_sb[j][:, 512 * ot : 512 * (ot + 1)], pp
                        )

                def attention_tile(tt):
                    nj = NJ_TILE[tt]
                    masked = set(_masked_js(tt))
                    pt_tiles = []
                    offs = []
                    for j in range(nj):
                        # in a masked (diagonal-region) block, the first
                        # 128*(j%4) gathered q-columns are fully masked out —
                        # skip computing them entirely
                        off = 128 * (j % 4) if j in masked else 0
                        offs.append(off)
                        sp = ps.tile([128, 512], F32, tag="score", bufs=4, name="score")
                        for o in range(OC):
                            nc.tensor.matmul(
                                sp[:, off:512],
                                lhsT=xt_sb[o][:, 128 * j : 128 * (j + 1)],
                                rhs=qp[tt][:, o % 2, o // 2, off:512],
                                start=(o == 0),
                                stop=(o == OC - 1),
                            )
                        pt = ptp.tile([128, 512], BF16, tag=f"pt{j}", name=f"pt{j}")
                        nc.scalar.activation(
                            pt[:, off:512],
                            sp[:, off:512],
                            mybir.ActivationFunctionType.Exp,
                            scale=SCALE,
                        )
                        if j in masked:
                            # only the diagonal 128-col q-block is partial;
                            # all columns beyond it are fully unmasked (and
                            # those before are skipped via `off`)
                            m = 4 * tt + (j % 4)
                            nc.vector.tensor_mul(
                                pt[:, off : off + 128],
                                pt[:, off : off + 128],
                                mask_all[:, m, off : off + 128],
                            )
                        pt_tiles.append(pt)

                    # partial softmax denominators: ones^T @ pt accumulated over j
                    rsp = ps.tile([1, 512], F32, tag="rs", bufs=1, name="rsp")
                    for j in range(nj):
                        nc.tensor.matmul(
                            rsp[:, offs[j] : 512],
                            lhsT=ones_sb,
                            rhs=pt_tiles[j][:, offs[j] : 512],
                            start=(j == 0),
                            stop=(j == nj - 1),
                        )
                    nc.vector.tensor_copy(rs_sb[:, 512 * tt : 512 * (tt + 1)], rsp)

                    # ascending: AV for qq needs pt blocks j < njs(qq), so low
                    # qq overlaps the tail of the exp/mask chain
                    for qq in (0, 1, 2, 3):
                        qbg = 4 * tt + qq        # gathered q-block index
                        njs = (qbg % 8) + 1      # causal chunk count in gathered order
                        ost = stg.tile([128, D], F32, tag="ost", name="ost")
                        for ot in range(2):
                            apsum = ps.tile(
                                [128, 512], F32, tag="attn", bufs=3, name="attn"
                            )
                            for j in range(njs):
                                nc.tensor.matmul(
                                    apsum,
                                    lhsT=pt_tiles[j][:, 128 * qq : 128 * (qq + 1)],
                                    rhs=vn_sb[j][:, 512 * ot : 512 * (ot + 1)],
                                    start=(j == 0),
                                    stop=(j == njs - 1),
                                )
                            nc.vector.tensor_copy(ost[:, 512 * ot : 512 * (ot + 1)], apsum)
                        # output stores on gpsimd (free after the collective
                        # triggers) so scalar keeps exp latency low; the last
                        # tile alternates gpsimd/scalar so its 4MB drains on
                        # two rings after the final matmul (exps are done by
                        # then)
                        if tt == NT - 1 and qq % 2 == 1:
                            eng = nc.scalar
                        else:
                            eng = nc.gpsimd
                        eng.dma_start(
                            out_p[128 * qbg : 128 * (qbg + 1), :], ost
                        )
                # tiles 0/1 (own q rows) never wait on the AllGathers;
                # tiles 2/3 (partner pieces) have ~50-70us of PE cover
                vn_late()
                attention_tile(0)
                recover(0)
                attention_tile(1)
                recover(1)
                attention_tile(2)
                attention_tile(3)

            nc.gpsimd.dma_start(rs_out, rs_sb)

    nc.compile()
    return nc


def _get_module():
    global _module_cache
    if _module_cache is None:
        _module_cache = _build_module()
    return _module_cache


def _gathered_q(p, par):
    """Global q index for gathered position p on a parity-`par` core."""
    p = np.asarray(p)
    blk = p // 128
    own = blk < 8
    gb = np.where(own, 2 * blk + par, 2 * (blk - 8) + (1 - par))
    return 128 * gb + p % 128


def _host_masks(par: int) -> np.ndarray:
    """[NMSK*128, 512] bf16 causal masks in this core's gathered q order."""
    out = np.zeros((NMSK * 128, 512), dtype=np.float32)
    k = np.arange(128)[:, None]
    ql = np.arange(512)[None, :]
    for tt in range(NT):
        for idx, j in enumerate(_masked_js(tt)):
            m = 4 * tt + idx
            g = 2 * j + par  # global k-chunk of local chunk j
            q_global = _gathered_q(512 * tt + ql, par)
            out[128 * m : 128 * (m + 1), :] = (q_global >= 128 * g + k).astype(
                np.float32
            )
    return out.astype(ml_dtypes.bfloat16)


def kernel(x, Wq, Wk, Wv, _trace=False):
    global last_results
    nc = _get_module()

    bf = ml_dtypes.bfloat16

    # weight-only folding: scores = x (Wq^T Wk) x^T
    wm = np.ascontiguousarray((Wq.T @ Wk).astype(bf))
    wvT = np.ascontiguousarray(Wv.T).astype(bf)
    masks = [_host_masks(0), _host_masks(1)]

    # per-parity column selection: core owns global k-chunks {2j+par}
    own_cols = [
        (128 * (2 * np.arange(LC)[:, None] + par) + np.arange(128)[None, :]).reshape(-1)
        for par in range(2)
    ]

    xTb = {}  # (b, par) -> packed own-context x^T
    for b in range(B):
        for par in range(2):
            t = x[b].T[:, own_cols[par]].astype(bf)  # [D, S//2]
            # pack as contiguous (i, half) blocks of [128, 512]
            xTb[(b, par)] = np.ascontiguousarray(
                t.reshape(IC, 128, 2, 512).transpose(0, 2, 1, 3)
            ).reshape(IC * 2 * 128, 512)

    in_maps = []
    for c in range(8):
        b, par = c // 2, c % 2
        in_maps.append(
            {
                "xT": xTb[(b, par)],
                "wm": wm,
                "wvT": wvT,
                "msk": masks[par],
            }
        )

    kwargs = {}
    if _trace:
        kwargs["trace"] = True
    res = run_bass_kernel_spmd(nc, in_maps, core_ids=list(range(8)), **kwargs)
    last_results = res

    # rows come back in each core's own gathered order; invert per parity
    gath_row = []
    for par in range(2):
        gr = np.empty(S, dtype=np.int64)
        gr[_gathered_q(np.arange(S), par)] = np.arange(S)
        gath_row.append(gr)

    out = np.empty((B, S, D), dtype=np.float32)
    for b in range(B):
        rA = res.results[2 * b]
        rB = res.results[2 * b + 1]
        num = rA["out_p"][gath_row[0]] + rB["out_p"][gath_row[1]]
        den = rA["rs_out"][0][gath_row[0]] + rB["rs_out"][0][gath_row[1]]
        out[b] = num / den[:, None]
    return out
